# revision 1
# baseline (speedup 1.0000x reference)
"""ByteAddFFN Trainium2 kernel v2 (single-core SPMD program).

Math per item, per nibble-stage k (0..7, = 2*byte + side, side 0=lo 1=hi):
  logits la/ha etc. via segment-sum reduces; E[xy] = exp(100(la[x]+lb[y]-m))
  built on the PE via one-hot matmuls (2-way bf16 mantissa split of the
  logits, b-side carries the max shift), exp applied by ACT from PSUM with
  bf16 output; U[t]/S1/S1'/ZZ via a second PE matmul against a 19-col
  one-hot table (rotU comes free as shifted slices of U).
  Carry chain linearized: g=sig(100*d0), r=sig(100*d1) (d0=2*S1*zeta-1)
  and c_{k+1} = g_k + (r_k-g_k)*c_k is ONE tensor_tensor_scan per tail
  group; SRAW = (U + c*(rotU-U))*zeta, then the nested softmax with the
  max-free second level: e2 = exp(100*rz*(e1-1)), out = outer(ehi', elo).
  Output written bf16 (host casts back to f32).
"""
import numpy as np
import ml_dtypes

import concourse.bass as bass
import concourse.mybir as mybir

F32 = mybir.dt.float32
BF16 = mybir.dt.bfloat16
ALU = mybir.AluOpType
AX = mybir.AxisListType
ACT = mybir.ActivationFunctionType
BF = ml_dtypes.bfloat16

NSPLIT = 2
KR = 32 * NSPLIT  # E-matmul contraction rows


def build_consts():
    ident = np.eye(128, dtype=BF)
    identf = np.eye(128, dtype=np.float32)
    repl = np.zeros((KR, 256), BF)
    for m in range(256):
        for s in range(NSPLIT):
            repl[32 * s + (m >> 4), m] = 1.0
            repl[32 * s + 16 + (m & 15), m] = 1.0
    # table [128, 2, 19]: U one-hots 0:16, S1=16, S1'=17, ZZ=18
    table = np.zeros((128, 2, 19), BF)
    for c in range(2):
        for r in range(128):
            t = ((128 * c + r) >> 4) + ((128 * c + r) & 15)
            table[r, c, t % 16] = 1.0
            if t >= 16:
                table[r, c, 16] = 1.0
            if t >= 15:
                table[r, c, 17] = 1.0
            table[r, c, 18] = 1.0
    return {"c_ident": ident, "c_identf": identf, "c_repl": repl, "c_table": table}


def build_kernel(nc, tc, ctx, M, F=2, TAILST=2, reps=1,
                 gp_prefold=True, out_gp_frac=0.5, ut_flip=True):
    SLOTS = M // 128
    NST = SLOTS // F
    N = F * 128
    NSL = TAILST * F
    GR = SLOTS // NSL
    assert SLOTS % (TAILST * F) == 0

    a_d = nc.dram_tensor("a", (M, 4, 256), F32, kind="ExternalInput")
    b_d = nc.dram_tensor("b", (M, 4, 256), F32, kind="ExternalInput")
    id_d = nc.dram_tensor("c_ident", (128, 128), BF16, kind="ExternalInput")
    idf_d = nc.dram_tensor("c_identf", (128, 128), F32, kind="ExternalInput")
    repl_d = nc.dram_tensor("c_repl", (KR, 256), BF16, kind="ExternalInput")
    tab_d = nc.dram_tensor("c_table", (128, 2, 19), BF16, kind="ExternalInput")
    o_d = nc.dram_tensor("out", (M, 4, 256), BF16, kind="ExternalOutput")

    a_v = a_d.ap().rearrange("(p t) b c -> p t (b c)", p=128)
    b_v = b_d.ap().rearrange("(p t) b c -> p t (b c)", p=128)
    o_v = o_d.ap().rearrange("(p t) b c -> p t (b c)", p=128)

    cpool = ctx.enter_context(tc.tile_pool(name="consts", bufs=1))
    abp = ctx.enter_context(tc.tile_pool(name="ab", bufs=2))
    stp = ctx.enter_context(tc.tile_pool(name="st", bufs=2))
    qp = ctx.enter_context(tc.tile_pool(name="q", bufs=3))
    rhsp = ctx.enter_context(tc.tile_pool(name="rhs", bufs=2))
    esbp = ctx.enter_context(tc.tile_pool(name="esb", bufs=1))
    ul1p = ctx.enter_context(tc.tile_pool(name="ul1", bufs=3))
    tlp = ctx.enter_context(tc.tile_pool(name="tl", bufs=4))
    outp = ctx.enter_context(tc.tile_pool(name="outp", bufs=3))
    smallp = ctx.enter_context(tc.tile_pool(name="small", bufs=3))
    tpp = ctx.enter_context(tc.tile_pool(name="tp", bufs=1, space="PSUM"))
    epp = ctx.enter_context(tc.tile_pool(name="ep", bufs=1, space="PSUM"))
    utp = ctx.enter_context(tc.tile_pool(name="ut", bufs=2, space="PSUM"))
    tbp = ctx.enter_context(tc.tile_pool(name="tb", bufs=2, space="PSUM"))
    utsp = ctx.enter_context(tc.tile_pool(name="utsb", bufs=3))

    ident = cpool.tile([128, 128], BF16)
    nc.sync.dma_start(ident[:], id_d.ap())
    identf = cpool.tile([128, 128], F32)
    nc.sync.dma_start(identf[:], idf_d.ap())
    repl = cpool.tile([KR, 256], BF16)
    nc.sync.dma_start(repl[:], repl_d.ap())
    tabl = cpool.tile([128, 2, 19], BF16)
    nc.sync.dma_start(tabl[:], tab_d.ap())
    b100 = cpool.tile([128, 1], F32)
    nc.gpsimd.memset(b100[:], 100.0)
    b200 = cpool.tile([128, 1], F32)
    nc.gpsimd.memset(b200[:], 200.0)

    # ---------------- tail phase: one group of NSL slots ----------------
    def tail_phase(ul1g_, g):
        tail_a(ul1g_, g)
        tail_b(*tail_a_out.pop(g))

    tail_a_out = {}

    def tail_a(ul1g_, g):
        ul1 = ul1g_[:].rearrange("p t j s c -> p t (j s) c")
        U = ul1[:, :, :, 0:16]
        ZET = tlp.tile([128, NSL, 8], F32, tag="zet")
        nc.vector.reciprocal(ZET[:], ul1[:, :, :, 18])
        zb = ZET[:].unsqueeze(3)

        W = tlp.tile([128, NSL, 8, 2], F32, tag="w")
        nc.vector.tensor_tensor(
            W[:], ul1[:, :, :, 16:18], zb.broadcast_to((128, NSL, 8, 2)),
            ALU.mult)
        W2 = smallp.tile([128, NSL, 1], F32, tag="w2")
        nc.vector.tensor_tensor(
            W2[:], W[:, :, 0, 0:1], W[:, :, 0, 1:2], ALU.add)
        nc.vector.tensor_scalar_max(W2[:], W2[:], 0.564)
        nc.vector.tensor_scalar_max(
            W[:].rearrange("p t k c -> p (t k c)"),
            W[:].rearrange("p t k c -> p (t k c)"), 0.0638)
        EG = tlp.tile([128, NSL, 17], F32, tag="eg")
        nc.scalar.activation(
            EG[:, :, 0:16], W[:].rearrange("p t k c -> p t (k c)"),
            ACT.Exp, scale=-200.0, bias=b100[:])
        nc.scalar.activation(
            EG[:, :, 16:17], W2[:], ACT.Exp, scale=-200.0, bias=b200[:])
        GALL = tlp.tile([128, NSL, 17], F32, tag="gall")
        nc.vector.tensor_scalar_add(EG[:], EG[:], 1.0)
        SAB = tlp.tile([128, 2, NSL, 8], F32, tag="sab")
        nc.gpsimd.memset(SAB[:, 0, :, 0:2], 0.0)
        nc.gpsimd.memset(SAB[:, 1, :, 0:1], 1.0)
        # r-values (odd cols) into GALL; g-values for stages 1..6 straight
        # into the scan B operand; h (col 16) into B position 1
        nc.vector.reciprocal(GALL[:], EG[:])
        egk = EG[:, :, 0:16].rearrange("p t (k c) -> p t k c", c=2)
        nc.vector.reciprocal(SAB[:, 1, :, 2:8], egk[:, :, 1:7, 0])
        nc.vector.reciprocal(SAB[:, 1, :, 1:2], EG[:, :, 16:17])
        gk = GALL[:, :, 0:16].rearrange("p t (k c) -> p t k c", c=2)
        nc.vector.tensor_tensor(
            SAB[:, 0, :, 2:8], gk[:, :, 1:7, 1], SAB[:, 1, :, 2:8],
            ALU.subtract)
        X8 = tlp.tile([128, NSL, 8], F32, tag="x8")
        nc.vector.tensor_tensor_scan(
            X8[:].rearrange("p t k -> p (t k)"),
            SAB[:, 0].rearrange("p t k -> p (t k)"),
            SAB[:, 1].rearrange("p t k -> p (t k)"),
            0.0, ALU.mult, ALU.add)

        R = tlp.tile([128, NSL, 8, 16], F32, tag="r")
        nc.vector.tensor_tensor(
            R[:, :, :, 1:16], U[:, :, :, 0:15], U[:, :, :, 1:16],
            ALU.subtract)
        nc.vector.tensor_tensor(
            R[:, :, :, 0:1], U[:, :, :, 15:16], U[:, :, :, 0:1],
            ALU.subtract)
        SRW = tlp.tile([128, NSL, 8, 16], F32, tag="srw")
        srwf = SRW[:].rearrange("p t k c -> p (t k) c")
        nc.gpsimd.tensor_tensor(
            srwf, R[:].rearrange("p t k c -> p (t k) c"),
            X8[:].rearrange("p t k -> p (t k)").unsqueeze(2)
            .broadcast_to((128, NSL * 8, 16)), ALU.mult)
        nc.gpsimd.tensor_tensor(
            srwf, srwf, U.rearrange("p t k c -> p (t k) c"), ALU.add)
        nc.vector.tensor_tensor(
            SRW[:, :, 0, 1:16], U[:, :, 0, 1:16], U[:, :, 0, 0:15], ALU.add)
        nc.vector.tensor_tensor(
            SRW[:, :, 0, 0:1], U[:, :, 0, 0:1], U[:, :, 0, 15:16], ALU.add)
        nc.gpsimd.tensor_tensor(
            srwf, srwf,
            ZET[:].rearrange("p t k -> p (t k)").unsqueeze(2)
            .broadcast_to((128, NSL * 8, 16)), ALU.mult)

        tail_a_out[g] = (SRW, ZET, g)

    def tail_b(SRW, ZET, g):
        srwf = SRW[:].rearrange("p t k c -> p (t k) c")
        M1 = tlp.tile([128, NSL, 8], F32, tag="m1")
        nc.vector.tensor_reduce(M1[:], srwf, AX.X, ALU.max, negate=True)
        nc.gpsimd.tensor_tensor(
            srwf, srwf,
            M1[:].rearrange("p t k -> p (t k)").unsqueeze(2)
            .broadcast_to((128, NSL * 8, 16)), ALU.add)
        E1 = tlp.tile([128, NSL, 8, 16], F32, tag="e1")
        e1f = E1[:].rearrange("p t k c -> p (t k) c")
        nc.scalar.activation(e1f, srwf, ACT.Exp, scale=100.0)
        Z1 = tlp.tile([128, NSL, 8], F32, tag="z1")
        nc.vector.tensor_reduce(Z1[:], e1f, AX.X, ALU.add)
        RZ = tlp.tile([128, NSL, 8], F32, tag="rz")
        nc.vector.reciprocal(RZ[:], Z1[:])
        nc.vector.scalar_tensor_tensor(
            e1f, e1f, 1.0,
            RZ[:].rearrange("p t k -> p (t k)").unsqueeze(2)
            .broadcast_to((128, NSL * 8, 16)),
            ALU.subtract, ALU.mult)
        E2 = tlp.tile([128, NSL, 8, 16], BF16, tag="e2")
        e2f = E2[:].rearrange("p t k c -> p (t k) c")
        nc.scalar.activation(e2f, e1f, ACT.Exp, scale=100.0)
        Z2 = tlp.tile([128, NSL, 8], F32, tag="z2")
        nc.vector.tensor_reduce(Z2[:], e2f, AX.X, ALU.add)
        z2v = Z2[:].rearrange("p t (j s) -> p t j s", s=2)
        RZZ = smallp.tile([128, NSL, 4], F32, tag="rzz")
        nc.vector.tensor_tensor(
            RZZ[:], z2v[:, :, :, 0], z2v[:, :, :, 1], ALU.mult)
        nc.vector.reciprocal(RZZ[:], RZZ[:])
        e2v = E2[:].rearrange("p t (j s) c -> p t j s c", s=2)
        EHI = tlp.tile([128, NSL, 4, 16], BF16, tag="ehi")
        nc.vector.tensor_tensor(
            EHI[:].rearrange("p t j c -> p (t j) c"),
            e2v[:, :, :, 1, :].rearrange("p t j c -> p (t j) c"),
            RZZ[:].rearrange("p t j -> p (t j)").unsqueeze(2)
            .broadcast_to((128, NSL * 4, 16)), ALU.mult)
        OT = outp.tile([128, NSL, 4, 16, 16], BF16, tag="ot")
        n_gp = int(round(out_gp_frac * NSL))
        for t in range(NSL):
            eng = nc.gpsimd if t < n_gp else nc.vector
            eng.tensor_tensor(
                OT[:, t], EHI[:, t].unsqueeze(3).broadcast_to(
                    (128, 4, 16, 16)),
                e2v[:, t, :, 0, :].unsqueeze(2).broadcast_to(
                    (128, 4, 16, 16)), ALU.mult)
        nc.sync.dma_start(
            o_v[:, g * NSL:(g + 1) * NSL, :],
            OT[:].rearrange("p t j x y -> p t (j x y)"))

    # ---------------- main loop ----------------
    for _rep in range(reps):
        ul1g = None
        atg = btg = None
        pending = []
        tail_lag = 0
        for s in range(NST):
            if s % TAILST == 0:
                ul1g = ul1p.tile([128, NSL, 4, 2, 32], F32, tag="ul1")
            atg = abp.tile([128, F, 4, 256], F32, tag="at")
            nc.sync.dma_start(atg[:], a_v[:, s * F:(s + 1) * F, :])
            btg = abp.tile([128, F, 4, 256], F32, tag="bt")
            nc.sync.dma_start(btg[:], b_v[:, s * F:(s + 1) * F, :])
            at = atg[:]
            bt = btg[:]

            st = stp.tile([128, F, 4, 64], F32, tag="stf")
            av = at.rearrange("p f b (h l) -> p (f b) h l", h=16, l=16)
            bv = bt.rearrange("p f b (h l) -> p (f b) h l", h=16, l=16)
            stv = st[:].rearrange("p f b c -> p (f b) c")

            for (xv, lo_off, hi_off, qtag) in (
                    (av, 0, 32, "qa"), (bv, 16, 48, "qb")):
                nc.vector.tensor_reduce(
                    stv[:, :, hi_off:hi_off + 16], xv, AX.X, ALU.add)
                if gp_prefold:
                    q = qp.tile([128, F * 4, 8, 16], F32, tag=qtag)
                    xw = xv.rearrange("p g h l -> p g (h l)").rearrange(
                        "p g (hp two l) -> p g hp two l", two=2, l=16)
                    nc.gpsimd.tensor_tensor(
                        q[:], xw[:, :, :, 0, :], xw[:, :, :, 1, :], ALU.add)
                    nc.vector.tensor_reduce(
                        stv[:, :, lo_off:lo_off + 16],
                        q[:].rearrange("p g hp l -> p g l hp"), AX.X, ALU.add)
                else:
                    nc.vector.tensor_reduce(
                        stv[:, :, lo_off:lo_off + 16],
                        xv.rearrange("p g h l -> p g l h"), AX.X, ALU.add)

            nma = smallp.tile([128, F * 4, 4], F32, tag="nma")
            nm = smallp.tile([128, F * 4, 2], F32, tag="nm")
            nc.vector.tensor_reduce(
                nma[:], stv.rearrange("p g (q c) -> p g q c", q=4),
                AX.X, ALU.max, negate=True)
            nc.vector.tensor_tensor(
                nm[:], nma[:, :, 0::2], nma[:, :, 1::2], ALU.add)
            stsh = stv.rearrange("p g (s c) -> p g s c", s=2)[:, :, :, 16:32]
            nc.gpsimd.tensor_tensor(
                stsh, stsh,
                nm[:].unsqueeze(3).broadcast_to((128, F * 4, 2, 16)), ALU.add)

            # 2-way bf16 split
            stb = stp.tile([128, F, 4, 2, KR], BF16, tag="stb")
            sb = stb[:].rearrange("p f b s c -> p (f b) s c")
            vb = stv.rearrange("p g (s c) -> p g s c", s=2)
            r1 = smallp.tile([128, F * 4, 2, 32], F32, tag="r1")
            nc.scalar.copy(sb[:, :, :, 0:32], vb)
            nc.vector.tensor_tensor(r1[:], vb, sb[:, :, :, 0:32], ALU.subtract)
            nc.gpsimd.tensor_copy(sb[:, :, :, 32:64], r1[:])

            # transpose each (f, j, side) [128, KR] -> [KR, 128]
            rhs = rhsp.tile([KR, 2, 4, F, 128], BF16, tag="rhs")
            for f in range(F):
                tp = tpp.tile([KR, 2, 4, 128], BF16, tag="tp")
                for si in range(2):
                    for j in range(4):
                        nc.tensor.transpose(
                            tp[:, si, j, :], stb[:, f, j, si, :], ident[:])
                nc.scalar.copy(rhs[:, :, :, f, :], tp[:])

            # E matmuls: per chunk c, weights loaded once; exp -> esb bf16
            esb = esbp.tile([128, 2, 2, 4, F, 128], BF16, tag="esb")
            for c in range(2):
                for side in range(2):
                    ep = epp.tile([128, 4, F, 128], F32, tag="ep")
                    rv = rhs[:, side, :, :, :].rearrange(
                        "k j f n -> k (j f n)")
                    ev = ep[:].rearrange("p j f n -> p (j f n)")
                    for q in range(2):
                        nc.tensor.matmul(
                            ev[:, q * 512:(q + 1) * 512],
                            repl[:, c * 128:(c + 1) * 128],
                            rv[:, q * 512:(q + 1) * 512])
                    nc.scalar.activation(
                        esb[:, c, side].rearrange("p j f n -> p (j f n)"),
                        ev, ACT.Exp, scale=100.0)

            # UT matmuls: table stationary, 4 j-groups packed into PSUM
            # partition quadrants via tile_position; xy-major result.
            if not ut_flip:
                for f in range(F):
                    up = utp.tile([128, 8, 19], F32, tag="ut")
                    for j in range(4):
                        for side in range(2):
                            k = 2 * j + side
                            for c in range(2):
                                nc.tensor.matmul(
                                    up[:, k, :],
                                    esb[:, c, side, j, f, :],
                                    tabl[:, c, :],
                                    start=(c == 0), stop=(c == 1))
                    slot = (s % TAILST) * F + f
                    dst = ul1g[:, slot].rearrange("p j s c -> p (j s) c")
                    nc.scalar.copy(dst[:, :, 0:19], up[:])
                if s % TAILST == TAILST - 1:
                    pending.append((ul1g, s // TAILST))
                if len(pending) > tail_lag:
                    tail_phase(*pending.pop(0))
                continue
            uts = utsp.tile([128, 2, F, 128], F32, tag="uts")
            for side in range(2):
                ut = utp.tile([128, F, 128], F32, tag="ut")
                for j in range(4):
                    for c in range(2):
                        nc.tensor.matmul(
                            ut[32 * j:32 * j + 19, :, :],
                            tabl[:, c, :],
                            esb[:, c, side, j, :, :],
                            start=(c == 0), stop=(c == 1),
                            tile_position=(0, 32 * j))
                nc.scalar.copy(uts[:, side, :, :], ut[:])
            # transpose back to item-major: [128 items, (4j x 32cols)] bf16
            for side in range(2):
                for f in range(F):
                    tb = tbp.tile([128, 128], F32, tag="tb")
                    nc.tensor.transpose(tb[:], uts[:, side, f, :], identf[:])
                    nc.scalar.copy(
                        ul1g[:, (s % TAILST) * F + f, :, side, :],
                        tb[:].rearrange("p (j c) -> p j c", c=32))

            if s % TAILST == TAILST - 1:
                pending.append((ul1g, s // TAILST))
            if pending and pending[0][1] not in tail_a_out:
                tail_a(*pending[0])
            if len(pending) > tail_lag:
                a = pending.pop(0)
                tail_b(*tail_a_out.pop(a[1]))
        for args in pending:
            if args[1] not in tail_a_out:
                tail_a(*args)
            tail_b(*tail_a_out.pop(args[1]))


# ======================================================================
# Runner: shard across 8 NeuronCores, compile once, execute via PJRT/axon
# ======================================================================
N_CORES = 8
_CACHE = {}


def _get_compiled(m_per_core, f=2, reps=1):
    key = (m_per_core, f, reps)
    if key not in _CACHE:
        from contextlib import ExitStack
        import concourse.bacc as bacc
        import concourse.tile as tile
        nc = bacc.Bacc("TRN2", target_bir_lowering=False, debug=False)
        with tile.TileContext(nc) as tc:
            with ExitStack() as ctx:
                build_kernel(nc, tc, ctx, m_per_core, F=f, reps=reps)
        nc.compile()
        _CACHE[key] = nc
    return _CACHE[key]


def kernel(a, b, b2n=None, n2b=None, add_table=None, carry_table=None,
           **_ignored):
    """Full-input entry point: a, b [32768, 4, 256] f32 -> out f32 same shape.

    Shards the batch across the 8 visible NeuronCores (pure data parallel),
    runs the Bass kernel SPMD, and concatenates the per-core outputs
    (device writes bf16; host upcasts to f32).
    """
    from concourse.bass_utils import run_bass_kernel_spmd

    a = np.ascontiguousarray(np.asarray(a, dtype=np.float32))
    b = np.ascontiguousarray(np.asarray(b, dtype=np.float32))
    B = a.shape[0]
    assert B % N_CORES == 0, f"batch {B} not divisible by {N_CORES} cores"
    mpc = B // N_CORES

    nc = _get_compiled(mpc)
    consts = build_consts()
    in_maps = []
    for c in range(N_CORES):
        m = {"a": a[c * mpc:(c + 1) * mpc], "b": b[c * mpc:(c + 1) * mpc]}
        m.update(consts)
        in_maps.append(m)
    res = run_bass_kernel_spmd(nc, in_maps, core_ids=list(range(N_CORES)))
    out = np.concatenate(
        [np.asarray(res.results[c]["out"]).astype(np.float32)
         for c in range(N_CORES)], axis=0)
    return out

